# revision 26
# baseline (speedup 1.0000x reference)
"""ConditionedPNA kernel for 8 trn2 NeuronCores.

Split: host does the data-dependent sparse selection (top-k nodes, valid-edge
selection, message gather, segment reductions -> mean/mx/mn/std aggregates);
the 8 NeuronCores (row-sharded over the 50176 padded nodes) do the dense PNA
update (3 matmul chains, degree-scaler combine, masked hidden update) AND the
score MLP.  hidden/score live on-device between layers (threaded state); only
hidden comes back (needed for exact f32 message values) plus the 200KB score.
The four batches are independent chains and are interleaved so device
transfers/exec overlap host numpy of the other batches.

PNA identity used: with scal = (1, amp, att) per node,
  out = concat(agg_a * scal_s) @ W  =  sum_s scal_s * (sum_a agg_a @ W[a,s])
so the 12-block feature concat never materializes anywhere.
"""
import os
import sys

sys.path.insert(0, "/opt/trn_rl_repo")

import numpy as np

# ---------------- problem constants (hardcoded per spec) ----------------
B, N, E, D, R2, T, M, L = 4, 50000, 1600000, 64, 1000, 32, 10000, 3
K = int(0.1 * N)                 # 5000
ESEL = int(1.0 * K * E / N)      # 160000
NCORES = 8
RPC = 6272                       # rows per core (49 * 128), 8*6272 = 50176 >= N
NPAD = NCORES * RPC
P = 128
NT = RPC // P                    # 49 tiles per core

_f32 = np.float32

# ---------------- device kernel (built lazily, cached) ----------------
_RUNNER = None


def _build_device():
    """Build the PNA-update+score bass kernel and a reusable 8-core runner."""
    import concourse.bass as bass
    import concourse.bacc as bacc
    import concourse.tile as tile
    from concourse import mybir
    from concourse.bass2jax import (
        install_neuronx_cc_hook,
        _bass_exec_p,
        partition_id_tensor,
    )
    import jax
    import jax.numpy as jnp
    from jax.sharding import Mesh, PartitionSpec
    from jax.experimental.shard_map import shard_map

    nc = bacc.Bacc(target_bir_lowering=False)
    dt = mybir.dt
    AF = mybir.ActivationFunctionType

    # per-core inputs (f32)
    mean_ = nc.dram_tensor("meanv", [RPC, D], dt.float32, kind="ExternalInput")
    mx_ = nc.dram_tensor("mxv", [RPC, D], dt.float32, kind="ExternalInput")
    mn_ = nc.dram_tensor("mnv", [RPC, D], dt.float32, kind="ExternalInput")
    std_ = nc.dram_tensor("stdv", [RPC, D], dt.float32, kind="ExternalInput")
    ampv = nc.dram_tensor("ampv", [RPC, 1], dt.float32, kind="ExternalInput")
    attv = nc.dram_tensor("attv", [RPC, 1], dt.float32, kind="ExternalInput")
    hasv = nc.dram_tensor("hasv", [RPC, 1], dt.float32, kind="ExternalInput")
    hasu = nc.dram_tensor("hasu", [RPC, 1], dt.uint8, kind="ExternalInput")
    hprev = nc.dram_tensor("hprev", [RPC, D], dt.float32, kind="ExternalInput")
    sprev = nc.dram_tensor("sprev", [RPC, 1], dt.float32, kind="ExternalInput")
    # weights: wc[:, (s*4+a)*64:+64] = pna_w[l] rows (a*3+s)*64:(a*3+s+1)*64
    wc = nc.dram_tensor("wc", [D, 12 * D], dt.float32, kind="ExternalInput")
    pnab = nc.dram_tensor("pnab", [1, 3 * D], dt.float32, kind="ExternalInput")
    lwh = nc.dram_tensor("lwh", [D, D], dt.float32, kind="ExternalInput")
    rowv = nc.dram_tensor("rowv", [1, D], dt.float32, kind="ExternalInput")
    m1 = nc.dram_tensor("m1", [D, 2 * D], dt.float32, kind="ExternalInput")
    b1 = nc.dram_tensor("b1", [1, 2 * D], dt.float32, kind="ExternalInput")
    m2 = nc.dram_tensor("m2", [2 * D, 1], dt.float32, kind="ExternalInput")
    b2 = nc.dram_tensor("b2", [1, 1], dt.float32, kind="ExternalInput")
    iden = nc.dram_tensor("iden", [128, 128], dt.float32, kind="ExternalInput")
    hnew = nc.dram_tensor("hnew", [RPC, D], dt.float32, kind="ExternalOutput")
    snew = nc.dram_tensor("snew", [RPC, 1], dt.float32, kind="ExternalOutput")

    with tile.TileContext(nc) as tc:
        with (
            tc.tile_pool(name="res", bufs=1) as res,
            tc.tile_pool(name="wk", bufs=3) as wk,
            tc.tile_pool(name="ps", bufs=1, space="PSUM") as ps,
        ):
            # resident weights
            wc_t = res.tile([D, 12 * D], dt.float32)
            nc.sync.dma_start(wc_t[:], wc[:])
            pnab_t = res.tile([1, 3 * D], dt.float32)
            nc.sync.dma_start(pnab_t[:], pnab[:])
            lwh_t = res.tile([D, D], dt.float32)
            nc.sync.dma_start(lwh_t[:], lwh[:])
            rowv_t = res.tile([1, D], dt.float32)
            nc.sync.dma_start(rowv_t[:], rowv[:])
            m1_t = res.tile([D, 2 * D], dt.float32)
            nc.sync.dma_start(m1_t[:], m1[:])
            b1_t = res.tile([1, 2 * D], dt.float32)
            nc.sync.dma_start(b1_t[:], b1[:])
            m2_t = res.tile([2 * D, 1], dt.float32)
            nc.sync.dma_start(m2_t[:], m2[:])
            b2_t = res.tile([1, 1], dt.float32)
            nc.sync.dma_start(b2_t[:], b2[:])
            iden_t = res.tile([128, 128], dt.float32)
            nc.sync.dma_start(iden_t[:], iden[:])
            ones1_t = res.tile([1, 128], dt.float32)
            nc.vector.memset(ones1_t[:], 1.0)

            for t in range(NT):
                lo = t * P
                hi = lo + P
                aggs = []
                for name, src in (("mean", mean_), ("mx", mx_), ("mn", mn_), ("std", std_)):
                    a_t = wk.tile([P, D], dt.float32, tag="agg_" + name)
                    nc.sync.dma_start(a_t[:], src[lo:hi, :])
                    aggs.append(a_t)
                amp_t = wk.tile([P, 1], dt.float32, tag="amp")
                nc.sync.dma_start(amp_t[:], ampv[lo:hi, :])
                att_t = wk.tile([P, 1], dt.float32, tag="att")
                nc.sync.dma_start(att_t[:], attv[lo:hi, :])
                has_t = wk.tile([P, 1], dt.float32, tag="has")
                nc.sync.dma_start(has_t[:], hasv[lo:hi, :])
                hasu_t = wk.tile([P, 1], dt.uint8, tag="hasu")
                nc.sync.dma_start(hasu_t[:], hasu[lo:hi, :])
                hp_t = wk.tile([P, D], dt.float32, tag="hp")
                nc.sync.dma_start(hp_t[:], hprev[lo:hi, :])
                sp_t = wk.tile([P, 1], dt.float32, tag="sp")
                nc.sync.dma_start(sp_t[:], sprev[lo:hi, :])

                # transpose the 4 aggregates: lhsT4[:, a*128:(a+1)*128] = agg_a^T
                lhsT4 = wk.tile([D, 4 * P], dt.float32, tag="lhsT4")
                for a in range(4):
                    psT = ps.tile([D, P], dt.float32, tag="psT")
                    nc.tensor.transpose(psT[:], aggs[a][:], iden_t[:])
                    nc.vector.tensor_copy(lhsT4[:, a * P:(a + 1) * P], psT[:])

                # 3 chains at once: psU [128, 192]; chain s cols s*64:(s+1)*64
                # rhs for agg a: wcr[:, a] rearranged so chains are adjacent.
                psU = ps.tile([P, 3 * D], dt.float32, tag="psU")
                for a in range(4):
                    nc.tensor.matmul(
                        psU[:],
                        lhsT4[:, a * P:(a + 1) * P],
                        wc_t[:, a * 3 * D:(a + 1) * 3 * D],
                        start=(a == 0),
                        stop=False,
                    )
                # bias row [1, 192]: pna_b in chain-0 cols, zeros elsewhere;
                # closes the whole accumulation region.
                nc.tensor.matmul(
                    psU[:], ones1_t[:], pnab_t[:], start=False, stop=True
                )

                # combine: upd = (U0 + amp*U1 + att*U2) * has ; hnew = hp + upd
                o1 = wk.tile([P, D], dt.float32, tag="o1")
                nc.vector.tensor_tensor(
                    o1[:], psU[:, D:2 * D], amp_t[:].to_broadcast([P, D]),
                    mybir.AluOpType.mult,
                )
                o2 = wk.tile([P, D], dt.float32, tag="o2")
                nc.vector.tensor_tensor(
                    o2[:], psU[:, 2 * D:3 * D], att_t[:].to_broadcast([P, D]),
                    mybir.AluOpType.mult,
                )
                nc.vector.tensor_add(o1[:], o1[:], psU[:, 0:D])
                nc.vector.tensor_add(o1[:], o1[:], o2[:])
                nc.vector.tensor_tensor(
                    o1[:], o1[:], has_t[:].to_broadcast([P, D]),
                    mybir.AluOpType.mult,
                )
                hn_t = wk.tile([P, D], dt.float32, tag="hn")
                nc.vector.tensor_add(hn_t[:], hp_t[:], o1[:])
                nc.sync.dma_start(hnew[lo:hi, :], hn_t[:])

                # ---- score MLP on hn ----
                psT2 = ps.tile([D, P], dt.float32, tag="psT2")
                nc.tensor.transpose(psT2[:], hn_t[:], iden_t[:])
                hnT = wk.tile([D, P], dt.float32, tag="hnT")
                nc.vector.tensor_copy(hnT[:], psT2[:])
                psH = ps.tile([P, D], dt.float32, tag="psH")
                nc.tensor.matmul(psH[:], hnT[:], lwh_t[:], start=True, stop=False)
                nc.tensor.matmul(psH[:], ones1_t[:], rowv_t[:], start=False, stop=True)
                x_t = wk.tile([P, D], dt.float32, tag="x")
                nc.vector.tensor_tensor(
                    x_t[:], hn_t[:], psH[:], mybir.AluOpType.mult
                )
                psT3 = ps.tile([D, P], dt.float32, tag="psT3")
                nc.tensor.transpose(psT3[:], x_t[:], iden_t[:])
                xT = wk.tile([D, P], dt.float32, tag="xT")
                nc.vector.tensor_copy(xT[:], psT3[:])
                psh1 = ps.tile([P, 2 * D], dt.float32, tag="psh1")
                nc.tensor.matmul(psh1[:], xT[:], m1_t[:], start=True, stop=False)
                nc.tensor.matmul(psh1[:], ones1_t[:], b1_t[:], start=False, stop=True)
                h1_t = wk.tile([P, 2 * D], dt.float32, tag="h1")
                nc.scalar.activation(h1_t[:], psh1[:], AF.Relu)
                psT4 = ps.tile([2 * D, P], dt.float32, tag="psT4")
                nc.tensor.transpose(psT4[:], h1_t[:], iden_t[:])
                h1T = wk.tile([2 * D, P], dt.float32, tag="h1T")
                nc.vector.tensor_copy(h1T[:], psT4[:])
                pss = ps.tile([P, 1], dt.float32, tag="pss")
                nc.tensor.matmul(pss[:], h1T[:], m2_t[:], start=True, stop=False)
                nc.tensor.matmul(pss[:], ones1_t[:], b2_t[:], start=False, stop=True)
                sn_t = wk.tile([P, 1], dt.float32, tag="sn")
                nc.vector.tensor_copy(sn_t[:], sp_t[:])
                nc.vector.copy_predicated(sn_t[:], hasu_t[:], pss[:])
                nc.sync.dma_start(snew[lo:hi, :], sn_t[:])
    nc.finalize()

    # ---- build a reusable jitted 8-core runner
    install_neuronx_cc_hook()
    from concourse import mybir as mb

    partition_name = nc.partition_id_tensor.name if nc.partition_id_tensor else None
    in_names, out_names, out_avals = [], [], []
    for alloc in nc.m.functions[0].allocations:
        if not isinstance(alloc, mb.MemoryLocationSet):
            continue
        name = alloc.memorylocations[0].name
        if alloc.kind == "ExternalInput":
            if name != partition_name:
                in_names.append(name)
        elif alloc.kind == "ExternalOutput":
            out_names.append(name)
            shape = tuple(alloc.tensor_shape)
            dtype = mb.dt.np(alloc.dtype)
            out_avals.append(jax.core.ShapedArray(shape, dtype))
    n_outs = len(out_avals)
    all_names = list(in_names) + list(out_names)
    if partition_name is not None:
        all_names.append(partition_name)

    def _body(*args):
        operands = list(args)
        if partition_name is not None:
            operands.append(partition_id_tensor())
        outs = _bass_exec_p.bind(
            *operands,
            out_avals=tuple(out_avals),
            in_names=tuple(all_names),
            out_names=tuple(out_names),
            lowering_input_output_aliases=(),
            sim_require_finite=True,
            sim_require_nnan=True,
            nc=nc,
        )
        return tuple(outs)

    devices = jax.devices()[:NCORES]
    mesh = Mesh(np.asarray(devices), ("core",))
    in_specs = (PartitionSpec("core"),) * (len(in_names) + n_outs)
    out_specs = (PartitionSpec("core"),) * n_outs
    sharded = jax.jit(
        shard_map(
            _body, mesh=mesh, in_specs=in_specs, out_specs=out_specs, check_rep=False
        ),
        keep_unused=True,
    )

    # output placeholder buffers: uploaded once, reused read-only every call
    # (outputs are fully DMA-written by the kernel, content never observed)
    from jax.sharding import NamedSharding

    zsh = NamedSharding(mesh, PartitionSpec("core"))
    placeholders = [
        jax.device_put(
            np.zeros((NCORES * av.shape[0], *av.shape[1:]), av.dtype), zsh
        )
        for av in out_avals
    ]

    def launch(named_inputs):
        """named_inputs: dict name -> full [NCORES*rows, ...] array (np or jax).
        Returns dict name -> lazy device array (full shape)."""
        args = [named_inputs[nm] for nm in in_names] + placeholders
        outs = sharded(*args)
        return dict(zip(out_names, outs))

    return launch


def _get_runner():
    global _RUNNER
    if _RUNNER is None:
        _RUNNER = _build_device()
    return _RUNNER


_ROW_GATHER = None


def _get_row_gather():
    """Jitted device-side row gather: (sharded [NPAD,D] array, idx) -> rows."""
    global _ROW_GATHER
    if _ROW_GATHER is None:
        import jax
        import jax.numpy as jnp

        _ROW_GATHER = jax.jit(lambda a, idx: jnp.take(a, idx, axis=0))
    return _ROW_GATHER


# ---------------- host-side exact helpers ----------------
def _sigmoid(x):
    x = x.astype(_f32)
    out = np.empty_like(x)
    pos = x >= 0
    out[pos] = (1.0 / (1.0 + np.exp(-x[pos]))).astype(_f32)
    ex = np.exp(x[~pos]).astype(_f32)
    out[~pos] = ex / (1.0 + ex)
    return out.astype(_f32)


def _score_fn(hidden, rel, linear_w, linear_b, mlp_w1, mlp_b1, mlp_w2, mlp_b2):
    """hidden [n,D], rel [D] -> [n], all float32."""
    heur = hidden @ linear_w[:D] + rel @ linear_w[D:] + linear_b
    x = hidden * heur
    h1 = np.maximum(x @ mlp_w1 + mlp_b1, 0.0)
    return (h1 @ mlp_w2 + mlp_b2).astype(_f32)[:, 0]


def _topk_sel(vals, k):
    """Boolean selection mask matching lax.top_k tie semantics
    (values desc, ties -> lowest index first)."""
    part = np.argpartition(-vals, k - 1)[:k]
    thr = vals[part].min()
    sel = vals > thr
    ngt = int(sel.sum())
    if ngt < k:
        eq = np.flatnonzero(vals == thr)[: k - ngt]
        sel[eq] = True
    return sel


def _pad_rows(x, rows):
    if x.ndim == 1:
        z = np.empty(rows, _f32)
        z[: x.shape[0]] = x
        z[x.shape[0]:] = 0.0
        return z[:, None]
    z = np.empty((rows, x.shape[1]), _f32)
    z[: x.shape[0]] = x
    z[x.shape[0]:] = 0.0
    return z


def kernel(h_index, r_index, t_index, all_index, edge_src, edge_dst, edge_type,
           hidden_states, score_text_embs, rel_table, linear_w, linear_b,
           mlp_w1, mlp_b1, mlp_w2, mlp_b2, relw, pna_w, pna_b):
    host_only = bool(os.environ.get("PNA_HOST_ONLY"))
    launch = None if host_only else _get_runner()

    h_index = np.asarray(h_index)
    r_index = np.asarray(r_index)
    t_index = np.asarray(t_index)
    all_index = np.asarray(all_index)
    edge_src = np.asarray(edge_src)
    edge_dst = np.asarray(edge_dst)
    edge_type = np.asarray(edge_type)
    hidden_states = np.asarray(hidden_states, dtype=_f32)
    score_text_embs = np.asarray(score_text_embs, dtype=_f32)
    rel_table = np.asarray(rel_table, dtype=_f32)
    linear_w = np.asarray(linear_w, dtype=_f32)
    linear_b = np.asarray(linear_b, dtype=_f32)
    mlp_w1 = np.asarray(mlp_w1, dtype=_f32)
    mlp_b1 = np.asarray(mlp_b1, dtype=_f32)
    mlp_w2 = np.asarray(mlp_w2, dtype=_f32)
    mlp_b2 = np.asarray(mlp_b2, dtype=_f32)
    relw = np.asarray(relw, dtype=_f32)
    pna_w = np.asarray(pna_w, dtype=_f32)
    pna_b = np.asarray(pna_b, dtype=_f32)

    deg_out_full = np.bincount(edge_src, minlength=N).astype(_f32)
    dmean = np.mean(np.log(deg_out_full + 1.0, dtype=_f32), dtype=_f32).astype(_f32)

    sf = lambda h, r: _score_fn(h, r, linear_w, linear_b, mlp_w1, mlp_b1, mlp_w2, mlp_b2)

    # ---- constant per-layer / per-batch device inputs (tiled per core) ----
    if not host_only:
        import jax
        from jax.sharding import Mesh, PartitionSpec, NamedSharding

        _mesh = Mesh(np.asarray(jax.devices()[:NCORES]), ("core",))
        _zsh = NamedSharding(_mesh, PartitionSpec("core"))

        def tile8(x):
            full = np.tile(np.ascontiguousarray(x, dtype=_f32), (NCORES, 1))
            return jax.device_put(full, _zsh)

        # wc layout: [:, a*192 + s*64 : +64] = pna_w[l] row-block (a*3+s)
        wcs = []
        for l in range(L):
            blocks = pna_w[l].reshape(12, D, D)  # block b = a*3+s
            wcl = np.empty((D, 12 * D), _f32)
            for a in range(4):
                for s in range(3):
                    wcl[:, a * 3 * D + s * D:(a * 3 + s + 1) * D] = blocks[a * 3 + s]
            wcs.append(tile8(wcl))
        pnabs = []
        for l in range(L):
            row = np.zeros((1, 3 * D), _f32)
            row[0, :D] = pna_b[l]
            pnabs.append(tile8(row))
        lwh8 = tile8(linear_w[:D])
        m18 = tile8(mlp_w1)
        b18 = tile8(mlp_b1[None, :])
        m28 = tile8(mlp_w2)
        b28 = tile8(mlp_b2.reshape(1, 1))
        iden8 = tile8(np.eye(128, dtype=_f32))
        rowvs = []
        for b in range(B):
            rel = rel_table[r_index[b]]
            rowvs.append(tile8((rel @ linear_w[D:] + linear_b)[None, :]))

    # ---- per-batch init (host) ----
    hidden0, score0 = [], []
    for b in range(B):
        rel = rel_table[r_index[b]]
        hidden = np.zeros((N, D), _f32)
        hidden[all_index] = score_text_embs
        hidden[h_index[b]] = hidden_states[b]
        base = sf(np.zeros((1, D), _f32), rel)[0]
        score = np.full(N, base, _f32)
        score[h_index[b]] = sf(hidden_states[b][None], rel)[0]
        hidden0.append(hidden)
        score0.append(score)

    def host_stage(score, row_lookup, l):
        """Selection + message aggregation; row_lookup(svs) returns the f32
        hidden rows for the (selected-src) node ids svs."""
        sel = _topk_sel(score, K)
        vidx = np.flatnonzero(sel[edge_src])
        if vidx.size > ESEL:
            es = score[edge_dst[vidx]]
            vsel = _topk_sel(es, ESEL)
            vidx = vidx[vsel]
        sv = edge_src[vidx]
        dv = edge_dst[vidx]
        et = edge_type[vidx]

        order = np.argsort(dv, kind="stable")
        svs, dvs, ets = sv[order], dv[order], et[order]
        gate = _sigmoid(score)
        ne = svs.shape[0]
        msg = row_lookup(svs, sel)
        msg *= gate[svs, None]
        msg *= np.take(relw[l], ets, axis=0)
        msg = msg.astype(_f32, copy=False)

        uniq, starts = np.unique(dvs, return_index=True)
        sm = np.zeros((N, D), _f32)
        sq = np.zeros((N, D), _f32)
        mx = np.zeros((N, D), _f32)
        mn = np.zeros((N, D), _f32)
        if len(uniq):
            # fused passes: one add-reduceat for (sum, sumsq), one
            # max-reduceat for (max, -min)
            xa = np.empty((ne, 2 * D), _f32)
            xa[:, :D] = msg
            np.square(msg, out=xa[:, D:])
            ra = np.add.reduceat(xa, starts, axis=0)
            sm[uniq] = ra[:, :D]
            sq[uniq] = ra[:, D:]
            xa[:, :D] = msg
            np.negative(msg, out=xa[:, D:])
            rb = np.maximum.reduceat(xa, starts, axis=0)
            mx[uniq] = rb[:, :D]
            np.negative(rb[:, D:], out=rb[:, D:])
            mn[uniq] = rb[:, D:]
        deg = np.bincount(dvs, minlength=N).astype(_f32)
        has = deg > 0.0
        degc = np.maximum(deg, 1.0)
        mean = (sm / degc[:, None]).astype(_f32)
        var = (sq / degc[:, None] - mean * mean).astype(_f32)
        std = np.where(has[:, None],
                       np.sqrt(np.maximum(var, 0.0) + _f32(1e-6), dtype=_f32),
                       0.0).astype(_f32)
        mx = np.where(has[:, None], mx, 0.0).astype(_f32)
        mn = np.where(has[:, None], mn, 0.0).astype(_f32)
        logd = np.log(deg + 1.0, dtype=_f32)
        ampa = (logd / dmean).astype(_f32)
        atta = np.where(has, dmean / np.maximum(logd, _f32(1e-6)), 0.0).astype(_f32)
        return mean, mx, mn, std, ampa, atta, has, deg

    out_scores = np.zeros((B, T), _f32)

    if host_only:
        for b in range(B):
            rel = rel_table[r_index[b]]
            hidden, score = hidden0[b], score0[b]
            for l in range(L):
                look = lambda svs, sel: np.take(hidden, svs, axis=0)
                mean, mx, mn, std, ampa, atta, has, deg = host_stage(score, look, l)
                one = np.ones_like(ampa)
                feats = np.concatenate(
                    [(a * sc[:, None]).astype(_f32)
                     for a in (mean, mx, mn, std) for sc in (one, ampa, atta)], -1)
                out = (feats @ pna_w[l] + pna_b[l]).astype(_f32)
                hidden = np.where(has[:, None], hidden + out, hidden).astype(_f32)
                news = sf(hidden, rel)
                score = np.where(deg > 0.0, news, score).astype(_f32)
            out_scores[b] = score[t_index[b]]
        return out_scores

    # ---- device-backed interleaved pipeline ----
    from concurrent.futures import ThreadPoolExecutor

    pool = ThreadPoolExecutor(1)

    def _prefetch(outs, want_hidden):
        # force dispatch/exec/d2h in the background; contiguous host copies
        sc = np.ascontiguousarray(np.asarray(outs["snew"])).ravel()[:N]
        hd = None
        if want_hidden:
            hd = np.ascontiguousarray(np.asarray(outs["hnew"])[:N])
        return sc, hd

    state = {}   # b -> (hprev array-like [NPAD,D], sprev [NPAD,1])
    pend = {}    # b -> future of score host array
    for b in range(B):
        state[b] = (_pad_rows(hidden0[b], NPAD), _pad_rows(score0[b], NPAD))

    for l in range(L):
        for b in range(B):
            if l == 0:
                score = score0[b]
                hid_cur = hidden0[b]
            else:
                score, hid_cur = pend[b].result()

            def look(svs, sel, hid_cur=hid_cur):
                nidx = np.flatnonzero(sel)
                rows = hid_cur[nidx]          # ~5000 rows, cache-resident
                return rows[np.searchsorted(nidx, svs)]

            mean, mx, mn, std, ampa, atta, has, deg = host_stage(score, look, l)
            named = {
                "meanv": _pad_rows(mean, NPAD),
                "mxv": _pad_rows(mx, NPAD),
                "mnv": _pad_rows(mn, NPAD),
                "stdv": _pad_rows(std, NPAD),
                "ampv": _pad_rows(ampa, NPAD),
                "attv": _pad_rows(atta, NPAD),
                "hasv": _pad_rows(has.astype(_f32), NPAD),
                "hasu": _pad_rows(has.astype(_f32), NPAD).astype(np.uint8),
                "hprev": state[b][0],
                "sprev": state[b][1],
                "wc": wcs[l],
                "pnab": pnabs[l],
                "lwh": lwh8,
                "rowv": rowvs[b],
                "m1": m18,
                "b1": b18,
                "m2": m28,
                "b2": b28,
                "iden": iden8,
            }
            outs = launch(named)
            pend[b] = pool.submit(_prefetch, outs, l < L - 1)
            state[b] = (outs["hnew"], outs["snew"])

    for b in range(B):
        score, _ = pend[b].result()
        out_scores[b] = score[t_index[b]]
    return out_scores


# revision 27
# speedup vs baseline: 1.0189x; 1.0189x over previous
"""ConditionedPNA kernel for 8 trn2 NeuronCores.

Split: host does the data-dependent sparse selection (top-k nodes, valid-edge
selection, message gather, segment reductions -> mean/mx/mn/std aggregates);
the 8 NeuronCores (row-sharded over the 50176 padded nodes) do the dense PNA
update (3 matmul chains, degree-scaler combine, masked hidden update) AND the
score MLP.  hidden/score live on-device between layers (threaded state); only
hidden comes back (needed for exact f32 message values) plus the 200KB score.
The four batches are independent chains and are interleaved so device
transfers/exec overlap host numpy of the other batches.

PNA identity used: with scal = (1, amp, att) per node,
  out = concat(agg_a * scal_s) @ W  =  sum_s scal_s * (sum_a agg_a @ W[a,s])
so the 12-block feature concat never materializes anywhere.
"""
import os
import sys

sys.path.insert(0, "/opt/trn_rl_repo")

import numpy as np

# ---------------- problem constants (hardcoded per spec) ----------------
B, N, E, D, R2, T, M, L = 4, 50000, 1600000, 64, 1000, 32, 10000, 3
K = int(0.1 * N)                 # 5000
ESEL = int(1.0 * K * E / N)      # 160000
NCORES = 8
RPC = 6272                       # rows per core (49 * 128), 8*6272 = 50176 >= N
NPAD = NCORES * RPC
P = 128
NT = RPC // P                    # 49 tiles per core

_f32 = np.float32

# ---------------- device kernel (built lazily, cached) ----------------
_RUNNER = None


def _build_device():
    """Build the PNA-update+score bass kernel and a reusable 8-core runner."""
    import concourse.bass as bass
    import concourse.bacc as bacc
    import concourse.tile as tile
    from concourse import mybir
    from concourse.bass2jax import (
        install_neuronx_cc_hook,
        _bass_exec_p,
        partition_id_tensor,
    )
    import jax
    import jax.numpy as jnp
    from jax.sharding import Mesh, PartitionSpec
    from jax.experimental.shard_map import shard_map

    nc = bacc.Bacc(target_bir_lowering=False)
    dt = mybir.dt
    AF = mybir.ActivationFunctionType

    # per-core inputs (f32)
    mean_ = nc.dram_tensor("meanv", [RPC, D], dt.float32, kind="ExternalInput")
    mx_ = nc.dram_tensor("mxv", [RPC, D], dt.float32, kind="ExternalInput")
    mn_ = nc.dram_tensor("mnv", [RPC, D], dt.float32, kind="ExternalInput")
    std_ = nc.dram_tensor("stdv", [RPC, D], dt.float32, kind="ExternalInput")
    ampv = nc.dram_tensor("ampv", [RPC, 1], dt.float32, kind="ExternalInput")
    attv = nc.dram_tensor("attv", [RPC, 1], dt.float32, kind="ExternalInput")
    hasv = nc.dram_tensor("hasv", [RPC, 1], dt.float32, kind="ExternalInput")
    hasu = nc.dram_tensor("hasu", [RPC, 1], dt.uint8, kind="ExternalInput")
    hprev = nc.dram_tensor("hprev", [RPC, D], dt.float32, kind="ExternalInput")
    sprev = nc.dram_tensor("sprev", [RPC, 1], dt.float32, kind="ExternalInput")
    # weights: wc[:, (s*4+a)*64:+64] = pna_w[l] rows (a*3+s)*64:(a*3+s+1)*64
    wc = nc.dram_tensor("wc", [D, 12 * D], dt.float32, kind="ExternalInput")
    pnab = nc.dram_tensor("pnab", [1, 3 * D], dt.float32, kind="ExternalInput")
    lwh = nc.dram_tensor("lwh", [D, D], dt.float32, kind="ExternalInput")
    rowv = nc.dram_tensor("rowv", [1, D], dt.float32, kind="ExternalInput")
    m1 = nc.dram_tensor("m1", [D, 2 * D], dt.float32, kind="ExternalInput")
    b1 = nc.dram_tensor("b1", [1, 2 * D], dt.float32, kind="ExternalInput")
    m2 = nc.dram_tensor("m2", [2 * D, 1], dt.float32, kind="ExternalInput")
    b2 = nc.dram_tensor("b2", [1, 1], dt.float32, kind="ExternalInput")
    iden = nc.dram_tensor("iden", [128, 128], dt.float32, kind="ExternalInput")
    hnew = nc.dram_tensor("hnew", [RPC, D], dt.float32, kind="ExternalOutput")
    snew = nc.dram_tensor("snew", [RPC, 1], dt.float32, kind="ExternalOutput")

    with tile.TileContext(nc) as tc:
        with (
            tc.tile_pool(name="res", bufs=1) as res,
            tc.tile_pool(name="wk", bufs=3) as wk,
            tc.tile_pool(name="ps", bufs=1, space="PSUM") as ps,
        ):
            # resident weights
            wc_t = res.tile([D, 12 * D], dt.float32)
            nc.sync.dma_start(wc_t[:], wc[:])
            pnab_t = res.tile([1, 3 * D], dt.float32)
            nc.sync.dma_start(pnab_t[:], pnab[:])
            lwh_t = res.tile([D, D], dt.float32)
            nc.sync.dma_start(lwh_t[:], lwh[:])
            rowv_t = res.tile([1, D], dt.float32)
            nc.sync.dma_start(rowv_t[:], rowv[:])
            m1_t = res.tile([D, 2 * D], dt.float32)
            nc.sync.dma_start(m1_t[:], m1[:])
            b1_t = res.tile([1, 2 * D], dt.float32)
            nc.sync.dma_start(b1_t[:], b1[:])
            m2_t = res.tile([2 * D, 1], dt.float32)
            nc.sync.dma_start(m2_t[:], m2[:])
            b2_t = res.tile([1, 1], dt.float32)
            nc.sync.dma_start(b2_t[:], b2[:])
            iden_t = res.tile([128, 128], dt.float32)
            nc.sync.dma_start(iden_t[:], iden[:])
            ones1_t = res.tile([1, 128], dt.float32)
            nc.vector.memset(ones1_t[:], 1.0)

            for t in range(NT):
                lo = t * P
                hi = lo + P
                aggs = []
                for name, src in (("mean", mean_), ("mx", mx_), ("mn", mn_), ("std", std_)):
                    a_t = wk.tile([P, D], dt.float32, tag="agg_" + name)
                    nc.sync.dma_start(a_t[:], src[lo:hi, :])
                    aggs.append(a_t)
                amp_t = wk.tile([P, 1], dt.float32, tag="amp")
                nc.sync.dma_start(amp_t[:], ampv[lo:hi, :])
                att_t = wk.tile([P, 1], dt.float32, tag="att")
                nc.sync.dma_start(att_t[:], attv[lo:hi, :])
                has_t = wk.tile([P, 1], dt.float32, tag="has")
                nc.sync.dma_start(has_t[:], hasv[lo:hi, :])
                hasu_t = wk.tile([P, 1], dt.uint8, tag="hasu")
                nc.sync.dma_start(hasu_t[:], hasu[lo:hi, :])
                hp_t = wk.tile([P, D], dt.float32, tag="hp")
                nc.sync.dma_start(hp_t[:], hprev[lo:hi, :])
                sp_t = wk.tile([P, 1], dt.float32, tag="sp")
                nc.sync.dma_start(sp_t[:], sprev[lo:hi, :])

                # transpose the 4 aggregates: lhsT4[:, a*128:(a+1)*128] = agg_a^T
                lhsT4 = wk.tile([D, 4 * P], dt.float32, tag="lhsT4")
                for a in range(4):
                    psT = ps.tile([D, P], dt.float32, tag="psT")
                    nc.tensor.transpose(psT[:], aggs[a][:], iden_t[:])
                    nc.vector.tensor_copy(lhsT4[:, a * P:(a + 1) * P], psT[:])

                # 3 chains at once: psU [128, 192]; chain s cols s*64:(s+1)*64
                # rhs for agg a: wcr[:, a] rearranged so chains are adjacent.
                psU = ps.tile([P, 3 * D], dt.float32, tag="psU")
                for a in range(4):
                    nc.tensor.matmul(
                        psU[:],
                        lhsT4[:, a * P:(a + 1) * P],
                        wc_t[:, a * 3 * D:(a + 1) * 3 * D],
                        start=(a == 0),
                        stop=False,
                    )
                # bias row [1, 192]: pna_b in chain-0 cols, zeros elsewhere;
                # closes the whole accumulation region.
                nc.tensor.matmul(
                    psU[:], ones1_t[:], pnab_t[:], start=False, stop=True
                )

                # combine: upd = (U0 + amp*U1 + att*U2) * has ; hnew = hp + upd
                o1 = wk.tile([P, D], dt.float32, tag="o1")
                nc.vector.tensor_tensor(
                    o1[:], psU[:, D:2 * D], amp_t[:].to_broadcast([P, D]),
                    mybir.AluOpType.mult,
                )
                o2 = wk.tile([P, D], dt.float32, tag="o2")
                nc.vector.tensor_tensor(
                    o2[:], psU[:, 2 * D:3 * D], att_t[:].to_broadcast([P, D]),
                    mybir.AluOpType.mult,
                )
                nc.vector.tensor_add(o1[:], o1[:], psU[:, 0:D])
                nc.vector.tensor_add(o1[:], o1[:], o2[:])
                nc.vector.tensor_tensor(
                    o1[:], o1[:], has_t[:].to_broadcast([P, D]),
                    mybir.AluOpType.mult,
                )
                hn_t = wk.tile([P, D], dt.float32, tag="hn")
                nc.vector.tensor_add(hn_t[:], hp_t[:], o1[:])
                nc.sync.dma_start(hnew[lo:hi, :], hn_t[:])

                # ---- score MLP on hn ----
                psT2 = ps.tile([D, P], dt.float32, tag="psT2")
                nc.tensor.transpose(psT2[:], hn_t[:], iden_t[:])
                hnT = wk.tile([D, P], dt.float32, tag="hnT")
                nc.vector.tensor_copy(hnT[:], psT2[:])
                psH = ps.tile([P, D], dt.float32, tag="psH")
                nc.tensor.matmul(psH[:], hnT[:], lwh_t[:], start=True, stop=False)
                nc.tensor.matmul(psH[:], ones1_t[:], rowv_t[:], start=False, stop=True)
                x_t = wk.tile([P, D], dt.float32, tag="x")
                nc.vector.tensor_tensor(
                    x_t[:], hn_t[:], psH[:], mybir.AluOpType.mult
                )
                psT3 = ps.tile([D, P], dt.float32, tag="psT3")
                nc.tensor.transpose(psT3[:], x_t[:], iden_t[:])
                xT = wk.tile([D, P], dt.float32, tag="xT")
                nc.vector.tensor_copy(xT[:], psT3[:])
                psh1 = ps.tile([P, 2 * D], dt.float32, tag="psh1")
                nc.tensor.matmul(psh1[:], xT[:], m1_t[:], start=True, stop=False)
                nc.tensor.matmul(psh1[:], ones1_t[:], b1_t[:], start=False, stop=True)
                h1_t = wk.tile([P, 2 * D], dt.float32, tag="h1")
                nc.scalar.activation(h1_t[:], psh1[:], AF.Relu)
                psT4 = ps.tile([2 * D, P], dt.float32, tag="psT4")
                nc.tensor.transpose(psT4[:], h1_t[:], iden_t[:])
                h1T = wk.tile([2 * D, P], dt.float32, tag="h1T")
                nc.vector.tensor_copy(h1T[:], psT4[:])
                pss = ps.tile([P, 1], dt.float32, tag="pss")
                nc.tensor.matmul(pss[:], h1T[:], m2_t[:], start=True, stop=False)
                nc.tensor.matmul(pss[:], ones1_t[:], b2_t[:], start=False, stop=True)
                sn_t = wk.tile([P, 1], dt.float32, tag="sn")
                nc.vector.tensor_copy(sn_t[:], sp_t[:])
                nc.vector.copy_predicated(sn_t[:], hasu_t[:], pss[:])
                nc.sync.dma_start(snew[lo:hi, :], sn_t[:])
    nc.finalize()

    # ---- build a reusable jitted 8-core runner
    install_neuronx_cc_hook()
    from concourse import mybir as mb

    partition_name = nc.partition_id_tensor.name if nc.partition_id_tensor else None
    in_names, out_names, out_avals = [], [], []
    for alloc in nc.m.functions[0].allocations:
        if not isinstance(alloc, mb.MemoryLocationSet):
            continue
        name = alloc.memorylocations[0].name
        if alloc.kind == "ExternalInput":
            if name != partition_name:
                in_names.append(name)
        elif alloc.kind == "ExternalOutput":
            out_names.append(name)
            shape = tuple(alloc.tensor_shape)
            dtype = mb.dt.np(alloc.dtype)
            out_avals.append(jax.core.ShapedArray(shape, dtype))
    n_outs = len(out_avals)
    all_names = list(in_names) + list(out_names)
    if partition_name is not None:
        all_names.append(partition_name)

    def _body(*args):
        operands = list(args)
        if partition_name is not None:
            operands.append(partition_id_tensor())
        outs = _bass_exec_p.bind(
            *operands,
            out_avals=tuple(out_avals),
            in_names=tuple(all_names),
            out_names=tuple(out_names),
            lowering_input_output_aliases=(),
            sim_require_finite=True,
            sim_require_nnan=True,
            nc=nc,
        )
        return tuple(outs)

    devices = jax.devices()[:NCORES]
    mesh = Mesh(np.asarray(devices), ("core",))
    in_specs = (PartitionSpec("core"),) * (len(in_names) + n_outs)
    out_specs = (PartitionSpec("core"),) * n_outs
    sharded = jax.jit(
        shard_map(
            _body, mesh=mesh, in_specs=in_specs, out_specs=out_specs, check_rep=False
        ),
        keep_unused=True,
    )

    # output placeholder buffers: uploaded once, reused read-only every call
    # (outputs are fully DMA-written by the kernel, content never observed)
    from jax.sharding import NamedSharding

    zsh = NamedSharding(mesh, PartitionSpec("core"))
    placeholders = [
        jax.device_put(
            np.zeros((NCORES * av.shape[0], *av.shape[1:]), av.dtype), zsh
        )
        for av in out_avals
    ]

    def launch(named_inputs):
        """named_inputs: dict name -> full [NCORES*rows, ...] array (np or jax).
        Returns dict name -> lazy device array (full shape)."""
        args = [named_inputs[nm] for nm in in_names] + placeholders
        outs = sharded(*args)
        return dict(zip(out_names, outs))

    return launch


def _get_runner():
    global _RUNNER
    if _RUNNER is None:
        _RUNNER = _build_device()
    return _RUNNER


_ROW_GATHER = None


def _get_row_gather():
    """Jitted device-side row gather: (sharded [NPAD,D] array, idx) -> rows."""
    global _ROW_GATHER
    if _ROW_GATHER is None:
        import jax
        import jax.numpy as jnp

        _ROW_GATHER = jax.jit(lambda a, idx: jnp.take(a, idx, axis=0))
    return _ROW_GATHER


# ---------------- host-side exact helpers ----------------
def _sigmoid(x):
    x = x.astype(_f32)
    out = np.empty_like(x)
    pos = x >= 0
    out[pos] = (1.0 / (1.0 + np.exp(-x[pos]))).astype(_f32)
    ex = np.exp(x[~pos]).astype(_f32)
    out[~pos] = ex / (1.0 + ex)
    return out.astype(_f32)


def _score_fn(hidden, rel, linear_w, linear_b, mlp_w1, mlp_b1, mlp_w2, mlp_b2):
    """hidden [n,D], rel [D] -> [n], all float32."""
    heur = hidden @ linear_w[:D] + rel @ linear_w[D:] + linear_b
    x = hidden * heur
    h1 = np.maximum(x @ mlp_w1 + mlp_b1, 0.0)
    return (h1 @ mlp_w2 + mlp_b2).astype(_f32)[:, 0]


def _topk_sel(vals, k):
    """Boolean selection mask matching lax.top_k tie semantics
    (values desc, ties -> lowest index first)."""
    part = np.argpartition(-vals, k - 1)[:k]
    thr = vals[part].min()
    sel = vals > thr
    ngt = int(sel.sum())
    if ngt < k:
        eq = np.flatnonzero(vals == thr)[: k - ngt]
        sel[eq] = True
    return sel


def _pad_rows(x, rows):
    if x.ndim == 1:
        z = np.empty(rows, _f32)
        z[: x.shape[0]] = x
        z[x.shape[0]:] = 0.0
        return z[:, None]
    z = np.empty((rows, x.shape[1]), _f32)
    z[: x.shape[0]] = x
    z[x.shape[0]:] = 0.0
    return z


def kernel(h_index, r_index, t_index, all_index, edge_src, edge_dst, edge_type,
           hidden_states, score_text_embs, rel_table, linear_w, linear_b,
           mlp_w1, mlp_b1, mlp_w2, mlp_b2, relw, pna_w, pna_b):
    host_only = bool(os.environ.get("PNA_HOST_ONLY"))
    launch = None if host_only else _get_runner()

    h_index = np.asarray(h_index)
    r_index = np.asarray(r_index)
    t_index = np.asarray(t_index)
    all_index = np.asarray(all_index)
    edge_src = np.asarray(edge_src)
    edge_dst = np.asarray(edge_dst)
    edge_type = np.asarray(edge_type)
    hidden_states = np.asarray(hidden_states, dtype=_f32)
    score_text_embs = np.asarray(score_text_embs, dtype=_f32)
    rel_table = np.asarray(rel_table, dtype=_f32)
    linear_w = np.asarray(linear_w, dtype=_f32)
    linear_b = np.asarray(linear_b, dtype=_f32)
    mlp_w1 = np.asarray(mlp_w1, dtype=_f32)
    mlp_b1 = np.asarray(mlp_b1, dtype=_f32)
    mlp_w2 = np.asarray(mlp_w2, dtype=_f32)
    mlp_b2 = np.asarray(mlp_b2, dtype=_f32)
    relw = np.asarray(relw, dtype=_f32)
    pna_w = np.asarray(pna_w, dtype=_f32)
    pna_b = np.asarray(pna_b, dtype=_f32)

    deg_out_full = np.bincount(edge_src, minlength=N).astype(_f32)
    dmean = np.mean(np.log(deg_out_full + 1.0, dtype=_f32), dtype=_f32).astype(_f32)

    sf = lambda h, r: _score_fn(h, r, linear_w, linear_b, mlp_w1, mlp_b1, mlp_w2, mlp_b2)

    # ---- constant per-layer / per-batch device inputs (tiled per core) ----
    if not host_only:
        import jax
        from jax.sharding import Mesh, PartitionSpec, NamedSharding

        _mesh = Mesh(np.asarray(jax.devices()[:NCORES]), ("core",))
        _zsh = NamedSharding(_mesh, PartitionSpec("core"))

        def tile8(x):
            full = np.tile(np.ascontiguousarray(x, dtype=_f32), (NCORES, 1))
            return jax.device_put(full, _zsh)

        # wc layout: [:, a*192 + s*64 : +64] = pna_w[l] row-block (a*3+s)
        wcs = []
        for l in range(L):
            blocks = pna_w[l].reshape(12, D, D)  # block b = a*3+s
            wcl = np.empty((D, 12 * D), _f32)
            for a in range(4):
                for s in range(3):
                    wcl[:, a * 3 * D + s * D:(a * 3 + s + 1) * D] = blocks[a * 3 + s]
            wcs.append(tile8(wcl))
        pnabs = []
        for l in range(L):
            row = np.zeros((1, 3 * D), _f32)
            row[0, :D] = pna_b[l]
            pnabs.append(tile8(row))
        lwh8 = tile8(linear_w[:D])
        m18 = tile8(mlp_w1)
        b18 = tile8(mlp_b1[None, :])
        m28 = tile8(mlp_w2)
        b28 = tile8(mlp_b2.reshape(1, 1))
        iden8 = tile8(np.eye(128, dtype=_f32))
        rowvs = []
        for b in range(B):
            rel = rel_table[r_index[b]]
            rowvs.append(tile8((rel @ linear_w[D:] + linear_b)[None, :]))

    # ---- per-batch init (host) ----
    hidden0, score0 = [], []
    for b in range(B):
        rel = rel_table[r_index[b]]
        hidden = np.zeros((N, D), _f32)
        hidden[all_index] = score_text_embs
        hidden[h_index[b]] = hidden_states[b]
        base = sf(np.zeros((1, D), _f32), rel)[0]
        score = np.full(N, base, _f32)
        score[h_index[b]] = sf(hidden_states[b][None], rel)[0]
        hidden0.append(hidden)
        score0.append(score)

    def host_stage(score, row_lookup, l):
        """Selection + message aggregation; row_lookup(svs) returns the f32
        hidden rows for the (selected-src) node ids svs."""
        sel = _topk_sel(score, K)
        vidx = np.flatnonzero(sel[edge_src])
        if vidx.size > ESEL:
            es = score[edge_dst[vidx]]
            vsel = _topk_sel(es, ESEL)
            vidx = vidx[vsel]
        sv = edge_src[vidx]
        dv = edge_dst[vidx]
        et = edge_type[vidx]

        order = np.argsort(dv, kind="stable")
        svs, dvs, ets = sv[order], dv[order], et[order]
        gate = _sigmoid(score)
        ne = svs.shape[0]
        msg = row_lookup(svs, sel)
        msg *= gate[svs, None]
        msg *= np.take(relw[l], ets, axis=0)
        msg = msg.astype(_f32, copy=False)

        uniq, starts = np.unique(dvs, return_index=True)
        sm = np.zeros((N, D), _f32)
        sq = np.zeros((N, D), _f32)
        mx = np.zeros((N, D), _f32)
        mn = np.zeros((N, D), _f32)
        if len(uniq):
            # fused passes: one add-reduceat for (sum, sumsq), one
            # max-reduceat for (max, -min)
            xa = np.empty((ne, 2 * D), _f32)
            xa[:, :D] = msg
            np.square(msg, out=xa[:, D:])
            ra = np.add.reduceat(xa, starts, axis=0)
            sm[uniq] = ra[:, :D]
            sq[uniq] = ra[:, D:]
            xa[:, :D] = msg
            np.negative(msg, out=xa[:, D:])
            rb = np.maximum.reduceat(xa, starts, axis=0)
            mx[uniq] = rb[:, :D]
            np.negative(rb[:, D:], out=rb[:, D:])
            mn[uniq] = rb[:, D:]
        deg = np.bincount(dvs, minlength=N).astype(_f32)
        has = deg > 0.0
        degc = np.maximum(deg, 1.0)
        mean = (sm / degc[:, None]).astype(_f32)
        var = (sq / degc[:, None] - mean * mean).astype(_f32)
        std = np.where(has[:, None],
                       np.sqrt(np.maximum(var, 0.0) + _f32(1e-6), dtype=_f32),
                       0.0).astype(_f32)
        mx = np.where(has[:, None], mx, 0.0).astype(_f32)
        mn = np.where(has[:, None], mn, 0.0).astype(_f32)
        logd = np.log(deg + 1.0, dtype=_f32)
        ampa = (logd / dmean).astype(_f32)
        atta = np.where(has, dmean / np.maximum(logd, _f32(1e-6)), 0.0).astype(_f32)
        return mean, mx, mn, std, ampa, atta, has, deg

    out_scores = np.zeros((B, T), _f32)

    if host_only:
        for b in range(B):
            rel = rel_table[r_index[b]]
            hidden, score = hidden0[b], score0[b]
            for l in range(L):
                look = lambda svs, sel: np.take(hidden, svs, axis=0)
                mean, mx, mn, std, ampa, atta, has, deg = host_stage(score, look, l)
                one = np.ones_like(ampa)
                feats = np.concatenate(
                    [(a * sc[:, None]).astype(_f32)
                     for a in (mean, mx, mn, std) for sc in (one, ampa, atta)], -1)
                out = (feats @ pna_w[l] + pna_b[l]).astype(_f32)
                hidden = np.where(has[:, None], hidden + out, hidden).astype(_f32)
                news = sf(hidden, rel)
                score = np.where(deg > 0.0, news, score).astype(_f32)
            out_scores[b] = score[t_index[b]]
        return out_scores

    # ---- device-backed interleaved pipeline ----
    from concurrent.futures import ThreadPoolExecutor

    pool = ThreadPoolExecutor(1)

    def _prefetch(outs, want_hidden):
        # force dispatch/exec/d2h in the background; contiguous host copies
        sc = np.ascontiguousarray(np.asarray(outs["snew"])).ravel()[:N]
        hd = None
        if want_hidden:
            hd = np.ascontiguousarray(np.asarray(outs["hnew"])[:N])
        return sc, hd

    state = {}   # b -> (hprev array-like [NPAD,D], sprev [NPAD,1])
    pend = {}    # b -> future of score host array
    for b in range(B):
        state[b] = (_pad_rows(hidden0[b], NPAD), _pad_rows(score0[b], NPAD))

    for l in range(L):
        for b in range(B):
            if l == 0:
                score = score0[b]
                hid_cur = hidden0[b]
            else:
                score, hid_cur = pend[b].result()

            def look(svs, sel, hid_cur=hid_cur):
                nidx = np.flatnonzero(sel)
                rows = hid_cur[nidx]          # ~5000 rows, cache-resident
                return rows[np.searchsorted(nidx, svs)]

            mean, mx, mn, std, ampa, atta, has, deg = host_stage(score, look, l)
            named = {
                "meanv": _pad_rows(mean, NPAD),
                "mxv": _pad_rows(mx, NPAD),
                "mnv": _pad_rows(mn, NPAD),
                "stdv": _pad_rows(std, NPAD),
                "ampv": _pad_rows(ampa, NPAD),
                "attv": _pad_rows(atta, NPAD),
                "hasv": _pad_rows(has.astype(_f32), NPAD),
                "hasu": _pad_rows(has.astype(_f32), NPAD).astype(np.uint8),
                "hprev": state[b][0],
                "sprev": state[b][1],
                "wc": wcs[l],
                "pnab": pnabs[l],
                "lwh": lwh8,
                "rowv": rowvs[b],
                "m1": m18,
                "b1": b18,
                "m2": m28,
                "b2": b28,
                "iden": iden8,
            }
            outs = launch(named)
            pend[b] = pool.submit(_prefetch, outs, l < L - 1)
            state[b] = (outs["hnew"], outs["snew"])

    for b in range(B):
        score, _ = pend[b].result()
        out_scores[b] = score[t_index[b]]
    pool.shutdown(wait=True)
    # release device buffers promptly so repeated calls don't accumulate
    for b in range(B):
        for arr in state[b]:
            if hasattr(arr, "delete"):
                try:
                    arr.delete()
                except Exception:
                    pass
    state.clear()
    pend.clear()
    return out_scores
